# revision 17
# baseline (speedup 1.0000x reference)
"""BlockCirculantLinear kernel for 8x TRN2 NeuronCores — FFT-domain einsum, v3.

Math: out = (x*D) @ M with M block-circulant (32x32 grid of 128-circulants).
Host does the cheap O(B d log b) rfft/irfft + packing; the device does the
frequency-domain einsum out_f = X_f @ V_f (a 32x32 complex matmul per bin).

Device kernel (per core, 1/8 of the batch = 1024 rows):
- Each bin's complex matmul is ONE dense 64x64 real matmul via the
  [[Re, Im], [-Im, Re]] block form: rhs = [XR; XI] (64 partitions),
  out = [YR; YI]. Four bins run concurrently on the four 64x64 quadrants
  of the PE array via tile_position; 128 MMs total.
- Input spectra ship as fp8 E3M4 (4 mantissa bits), scaled by 14/max
  on host; mixed-dtype matmul (bf16 lhsT x fp8e3 rhs) keeps weights
  full precision. Input HBM: 8MB -> 4.2MB per core.
- Output: the 32 lowest-energy bins (ranked by a per-bin energy proxy)
  are routed to the partition-half-1 slots and evacuated as fp8 E3M4
  (psum -> SBUF cast on the scalar engine); the 32 high-energy bins
  stay bf16 (vector engine). Per-bin scales, folded into the weights
  with a hard Cauchy-Schwarz bound (|psum| <= 12.9 < 15.5 max normal,
  overflow impossible), are divided back out on the host. Output HBM:
  8MB -> 6.3MB per core. End-to-end rel err ~1.6e-2 (gate 2e-2;
  all-bf16 is 3.1e-3, and all-fp8-out would be 1.92e-2 - too tight).
- PSUM is organized as 2-bank tiles [128, 2, 512] so each group needs
  one [128,1024] evacuation copy per engine (the v2 per-bank copies at
  ~0.7us each made the copy pipeline the critical path).
- DMA: strict FIFO on the Sync/SP HWDGE ring only: weights, 4x 1MB
  input units, then the output DMAs — reads get absolute priority,
  which is the makespan-optimal schedule at the ~360GB/s per-NC HBM
  cap. (v2 put the weights on the ACT ring, which started ~4us late
  and stalled the first real matmuls to 17.7us, HAM-cold.) Total
  ~11MB/core -> ~31us DMA window + ~9us NRT preamble + ~8us postamble.
- HAM pre-warm garbage MMs bridge until the first input unit lands.
"""

import numpy as np
import ml_dtypes

B_TOTAL = 8192
D_IN = 4096
D_OUT = 4096
BLK = 128
K_IN = D_IN // BLK    # 32
K_OUT = D_OUT // BLK  # 32
N_CORES = 8
B_SHARD = B_TOTAL // N_CORES  # 1024
NG = 16               # groups of 4 bins (64 plane-pairs)
NU = 8                # input DMA units (2 groups = 0.5MB each)
NPO = 8               # output DMA pairs (2 groups each)
MM_FREE = 512         # moving free dim per matmul (one PSUM bank)
XSCALE_TGT = 14.0     # fp8 e3m4 max normal is 15.5
YSCALE_TGT = 12.9

_compiled = None


def _build_module():
    import concourse.bass as bass
    import concourse.tile as tile
    from concourse import bacc, mybir

    nc = bacc.Bacc("TRN2", target_bir_lowering=False, debug=False)

    bf = mybir.dt.bfloat16
    f8 = mybir.dt.float8e3
    f32 = mybir.dt.float32

    # xq: flat; per unit of ng groups: [128, ng, 2, B] with contiguous
    # per-partition rows. p = 64*half + 32*comp + j; slot = (g, half, u)
    xq = nc.dram_tensor("xq", [NG * 128 * 2 * B_SHARD], f8, kind="ExternalInput")
    # wt[p, g, u, oc]: p = 64*half + jc; lhsT of the 64x64 bin matrix
    wt = nc.dram_tensor("wt", [128, NG, 2, 64], bf, kind="ExternalInput")
    # ybf[pair, p, gj, mc, m']: bank A (input-half-0 bins), bf16
    ybf = nc.dram_tensor("ybf", [NPO, 128, 2, 2, MM_FREE], bf, kind="ExternalOutput")
    # yq8[pair, p, gj, mc, m']: bank B (input-half-1 bins), fp8 e3m4
    yq8 = nc.dram_tensor("yq8", [NPO, 128, 2, 2, MM_FREE], f8, kind="ExternalOutput")

    U_ELEMS = 128 * 2 * 2 * B_SHARD   # 0.5MB fp8 per input unit
    PO_ELEMS = 128 * 2 * 2 * MM_FREE  # per output pair per tensor

    with tile.TileContext(nc) as tc:
        with (
            tc.tile_pool(name="sb", bufs=1) as spool,
            tc.tile_pool(name="psum", bufs=2, space="PSUM") as ppool,
        ):
            w = spool.tile([128, NG, 2, 64], bf, name="wt")
            # weights lead the Sync ring: land ~10.2us, before the first
            # real MMs need them (~12.6) — the ACT ring starts ~4us late
            nc.sync.dma_start(w[:], wt[:])

            scratch = spool.tile([128, MM_FREE], bf, name="scratch")
            nc.vector.memset(scratch[:], 0.0)

            # lead units of 1 group each so the first MMs (and the copy
            # pipeline, the end-game critical path) start ~0.5us earlier;
            # the rest arrive faster than the copy-bound pipeline consumes
            UNITS = [(0, 1), (1, 1), (2, 2), (4, 2), (6, 2), (8, 2), (10, 2), (12, 2), (14, 2)]
            G_ELEMS = 128 * 2 * B_SHARD
            xts = {}
            for ui, (g0, ng) in enumerate(UNITS):
                xt = spool.tile(
                    [128, ng, 2, B_SHARD], f8, tag=f"xt{ng}", name=f"xt{ui}",
                    bufs=(2 if ng == 1 else NU - 1),
                )
                row = ng * 2 * B_SHARD
                nc.sync.dma_start(
                    xt[:],
                    bass.AP(xq, g0 * G_ELEMS, [[row, 128], [1, row]]),
                )
                for k in range(ng):
                    xts[g0 + k] = (xt, k)

            otA = otB = None
            for g in range(NG):
                xt, gi = xts[g]
                # psA: bank-pair for input-half-0 bins (a: cols 0-63, b: 64-127)
                # psB: bank-pair for input-half-1 bins (c, d); [128, mc, 512]
                psA = ppool.tile([128, 2, MM_FREE], f32, tag="psA", name=f"psA_{g}")
                psB = ppool.tile([128, 2, MM_FREE], f32, tag="psB", name=f"psB_{g}")
                if g == 0:
                    # HAM pre-warm: garbage matmuls while the weights and the
                    # first input unit stream in (~3.4us of sustained PE
                    # activity trips the 2.4GHz un-throttle); results are
                    # wiped by the real MMs' start=True
                    for k in range(11):
                        nc.tensor.matmul(
                            (psA if k % 2 == 0 else psB)[:, k % 2, :],
                            lhsT=scratch[:, 0:128],
                            rhs=scratch[:],
                            start=True,
                            stop=True,
                        )
                elif g % 2 != 0 and g < NG - 2:
                    # keep-warm filler; insurance against input-DMA jitter
                    nc.tensor.matmul(
                        psA[:, 0, :],
                        lhsT=scratch[:, 0:128],
                        rhs=scratch[:],
                        start=True,
                        stop=True,
                    )
                for mc in range(2):
                    s = slice(mc * MM_FREE, (mc + 1) * MM_FREE)
                    for half, u, ps, tp in (
                        (0, 0, psA, (0, 0)),
                        (0, 1, psA, (0, 64)),
                        (1, 0, psB, (64, 0)),
                        (1, 1, psB, (64, 64)),
                    ):
                        rows = slice(64 * half, 64 * half + 64)
                        cols = slice(tp[1], tp[1] + 64)
                        nc.tensor.matmul(
                            ps[cols, mc, :],
                            lhsT=w[rows, g, u, :],
                            rhs=xt[rows, gi, u, s],
                            start=True,
                            stop=True,
                            tile_position=tp,
                        )

                pr, gj = g // 2, g % 2
                if g >= NG - 2:
                    # per-group tiles + DMAs for the last two groups: the
                    # exit-gating final bytes leave ~1us earlier
                    otA = spool.tile([128, 1, 2, MM_FREE], bf, tag="otAl", name=f"otAl{gj}", bufs=2)
                    otB = spool.tile([128, 1, 2, MM_FREE], f8, tag="otBl", name=f"otBl{gj}", bufs=2)
                    oA, oB = otA[:, 0], otB[:, 0]
                elif gj == 0:
                    otA = spool.tile([128, 2, 2, MM_FREE], bf, tag="otA", name=f"otA{pr}", bufs=NPO - 1)
                    otB = spool.tile([128, 2, 2, MM_FREE], f8, tag="otB", name=f"otB{pr}", bufs=NPO - 1)
                    oA, oB = otA[:, gj], otB[:, gj]
                else:
                    oA, oB = otA[:, gj], otB[:, gj]
                # evacuate PSUM: one [128,1024] copy per engine per group;
                # the copy pipeline is the end-game critical path (vector
                # 1.22us/copy, scalar 1.11): give scalar one extra pair
                if g == 7:
                    nc.scalar.copy(oA, psA[:])
                    nc.scalar.copy(oB, psB[:])
                else:
                    nc.vector.tensor_copy(oA, psA[:])
                    nc.scalar.copy(oB, psB[:])

                if g >= NG - 2:
                    nc.sync.dma_start(
                        bass.AP(ybf, pr * PO_ELEMS + gj * 1024, [[2048, 128], [1, 1024]]),
                        oA,
                    )
                    nc.sync.dma_start(
                        bass.AP(yq8, pr * PO_ELEMS + gj * 1024, [[2048, 128], [1, 1024]]),
                        oB,
                    )
                elif gj == 1:
                    nc.sync.dma_start(
                        bass.AP(ybf, pr * PO_ELEMS, [[2048, 128], [1, 2048]]),
                        otA[:],
                    )
                    nc.sync.dma_start(
                        bass.AP(yq8, pr * PO_ELEMS, [[2048, 128], [1, 2048]]),
                        otB[:],
                    )

    nc.compile()
    return nc


def _get_module():
    global _compiled
    if _compiled is None:
        _compiled = _build_module()
    return _compiled


def kernel(x: np.ndarray, W: np.ndarray, D_bernoulli: np.ndarray) -> np.ndarray:
    from concourse.bass_utils import run_bass_kernel_spmd

    bf16 = ml_dtypes.bfloat16
    e3m4 = ml_dtypes.float8_e3m4
    x = np.asarray(x, dtype=np.float32)
    W = np.asarray(W, dtype=np.float32)
    D = np.asarray(D_bernoulli, dtype=np.float32)

    # --- host: forward rfft of (x*D) blocks ---
    xd = (x * D[None, :]).reshape(B_TOTAL, K_IN, BLK)
    Xf = np.fft.rfft(xd, axis=-1)                 # [B, 32, 65]
    Xr = np.ascontiguousarray(Xf.real.transpose(2, 1, 0)).astype(np.float32)  # [65, 32, B]
    Xi = np.ascontiguousarray(Xf.imag.transpose(2, 1, 0)).astype(np.float32)
    XR = Xr[:64]                                  # [64 bins, 32 j, B]
    XI = Xi[:64].copy()
    XI[0] = Xr[64]                                # R64 rides in the I0 slot

    # fp8 e3m4 input scale; the weights absorb 1/s
    s = XSCALE_TGT / max(np.abs(XR).max(), np.abs(XI).max())
    XRq = (XR * s).astype(e3m4)
    XIq = (XI * s).astype(e3m4)

    # --- host: 64x64 bin matrices M2 = [[A,C],[B,D]] (lhsT) ---
    Vf = np.conj(np.fft.rfft(W, axis=-1))         # [o, j, 65]
    VR = np.ascontiguousarray(Vf.real.transpose(2, 1, 0)).astype(np.float32)  # [65, j, o]
    VI = np.ascontiguousarray(Vf.imag.transpose(2, 1, 0)).astype(np.float32)
    M2 = np.empty((64, 64, 64), dtype=np.float32)  # [bin, jc, oc]
    M2[:, :K_IN, :K_OUT] = VR[:64]                 # A  (YR += A.XR)
    M2[:, :K_IN, K_OUT:] = VI[:64]                 # C  (YI += C.XR)
    M2[:, K_IN:, :K_OUT] = -VI[:64]                # B  (YR += B.XI)
    M2[:, K_IN:, K_OUT:] = VR[:64]                 # D  (YI += D.XI)
    M2[0, :K_IN, K_OUT:] = 0.0                     # bin 0/64 are real-only
    M2[0, K_IN:, :K_OUT] = 0.0
    M2[0, K_IN:, K_OUT:] = VR[64]                  # R64 channel in the I0 slot

    # --- bin permutation: 32 lowest-energy bins -> fp8 output half ---
    XRf = XRq.astype(np.float32)
    XIf = XIq.astype(np.float32)
    PX = np.concatenate(
        [(XRf ** 2).mean(axis=2), (XIf ** 2).mean(axis=2)], axis=1
    )                                              # [64, 64] E[x2q^2] per jc
    proxy = np.einsum('fjo,fj->f', M2 ** 2, PX) / (s * s)
    wgt = np.full(64, 2.0); wgt[0] = 1.0
    order = np.argsort(proxy * wgt)
    lo_bins = np.sort(order[:32])                  # fp8 output half (half=1)
    hi_bins = np.sort(order[32:])                  # bf16 output half (half=0)
    slot_bin = np.empty((NG, 2, 2), dtype=np.int64)
    slot_bin[:, 0, :] = hi_bins.reshape(NG, 2)
    slot_bin[:, 1, :] = lo_bins.reshape(NG, 2)

    # --- per-bin output scales for the fp8 half (hard C-S bound) ---
    # |psum[oc]| <= ||(M2/s)[:,oc]|| * max_b ||x2q*s...|| ; rhs on device
    # is XRq/XIq (already scaled by s), lhsT is M2/s
    x2n = np.sqrt(
        (XRf ** 2).sum(axis=1) + (XIf ** 2).sum(axis=1)
    ).max(axis=1)                                  # [64] max_b ||x2q_b||
    coln = np.sqrt((M2 ** 2).sum(axis=1)).max(axis=1) / s   # [64] max_oc ||.||
    bound = coln * x2n                             # per-bin hard |psum| bound
    sigma = np.ones(64, dtype=np.float32)
    sigma[lo_bins] = YSCALE_TGT / bound[lo_bins]

    # --- weights -> wt[p, g, u, oc] per the slot map, scaled ---
    M2s = M2 * (sigma / s)[:, None, None]
    wt_host = np.empty((128, NG, 2, 64), dtype=bf16)
    for g in range(NG):
        for half in range(2):
            for u in range(2):
                wt_host[64 * half : 64 * half + 64, g, u, :] = (
                    M2s[slot_bin[g, half, u]].astype(bf16)
                )

    # --- pack inputs per the slot map into per-group blocks [g, p, u, m];
    # device units concatenate groups with per-partition-contiguous rows ---
    UNITS = [(0, 1), (1, 1), (2, 2), (4, 2), (6, 2), (8, 2), (10, 2), (12, 2), (14, 2)]
    Z = np.stack([XRq, XIq], axis=1)               # [64, 2(comp), 32, B]
    gblk = np.empty((NG, 2, 2, K_IN, 2, B_TOTAL), dtype=e3m4)
    # axes: [g, half, comp, j, u, m]; p = 64*half + 32*comp + j
    for g in range(NG):
        for half in range(2):
            for u in range(2):
                gblk[g, half, :, :, u, :] = Z[slot_bin[g, half, u]]
    gblk = gblk.reshape(NG, 128, 2, B_TOTAL)

    in_maps = []
    for c in range(N_CORES):
        sl = slice(c * B_SHARD, (c + 1) * B_SHARD)
        parts = []
        for g0, ng in UNITS:
            blk = gblk[g0 : g0 + ng, :, :, sl]     # [ng, 128, 2, 1024]
            parts.append(np.ascontiguousarray(blk.transpose(1, 0, 2, 3)).ravel())
        in_maps.append({"xq": np.concatenate(parts), "wt": wt_host})

    nc = _get_module()
    res = run_bass_kernel_spmd(nc, in_maps, core_ids=list(range(N_CORES)))

    # --- host: unpack spectra, irfft, reassemble ---
    inv_sigma = (1.0 / sigma).astype(np.float32)
    out = np.empty((B_TOTAL, D_OUT), dtype=np.float32)
    NB = BLK // 2 + 1
    for c in range(N_CORES):
        PS = np.empty((64, 2, K_OUT, B_SHARD), dtype=np.float32)  # [bin, comp, o, m]
        for half, key in ((0, "ybf"), (1, "yq8")):
            y = np.asarray(res.results[c][key], dtype=np.float32)  # [NPO,128,2,2,512]
            # y[pr, 64*oh + 32*comp + o, gj, mc, m'] ; bin slot (g=2pr+gj, half, u=oh)
            yb = y.reshape(NPO, 2, 2, K_OUT, 2, 2 * MM_FREE)  # [pr, oh, comp, o, gj, m]
            for pr in range(NPO):
                for gj in range(2):
                    for oh in range(2):
                        b = slot_bin[2 * pr + gj, half, oh]
                        PS[b] = yb[pr, oh, :, :, gj, :] * inv_sigma[b]
        psR, psI = PS[:, 0], PS[:, 1]                    # [64, o, m]
        Yf = np.zeros((B_SHARD, K_OUT, NB), dtype=np.complex64)
        Yf[:, :, :64] = (psR + 1j * psI).transpose(2, 1, 0)
        Yf[:, :, 0] = psR[0].T
        Yf[:, :, 64] = psI[0].T
        ob = np.fft.irfft(Yf, n=BLK, axis=-1)            # [m, 32, 128]
        out[c * B_SHARD : (c + 1) * B_SHARD] = ob.reshape(B_SHARD, D_OUT)
    return out


# revision 18
# speedup vs baseline: 1.0171x; 1.0171x over previous
"""BlockCirculantLinear kernel for 8x TRN2 NeuronCores — FFT-domain einsum, v6.

Math: out = (x*D) @ M with M block-circulant (32x32 grid of 128-circulants).
Host does the cheap O(B d log b) rfft/irfft + packing; the device does the
frequency-domain einsum out_f = X_f @ V_f (a 32x32 complex matmul per bin).

Device kernel (per core, 1/8 of the batch = 1024 rows):
- Each bin's complex matmul is ONE dense 64x64 real matmul via the
  [[Re, Im], [-Im, Re]] block form: rhs = [XR; XI] (64 partitions),
  out = [YR; YI]. Four bins run concurrently on the four 64x64 quadrants
  of the PE array via tile_position; 128 MMs total.
- Input spectra ship as fp8 E3M4 (4 mantissa bits), scaled by 14/max on
  host; mixed-dtype matmul (bf16 lhsT x fp8e3 rhs) keeps weights full
  precision. Input HBM: 8MB -> 4.2MB per core.
- Output: the 32 lowest-energy bins (per-bin energy proxy) are routed to
  the partition-half-1 slots and evacuated as fp8 E3M4; the 32 high-
  energy bins stay bf16. Per-bin scales are folded into the weights with
  a hard Cauchy-Schwarz bound (|psum| <= 12.9 < 15.5, overflow
  impossible) and divided back out on the host. Output HBM: 8MB ->
  6.3MB. End-to-end rel err 1.58e-2 (gate 2e-2; all-bf16 is 3.1e-3).
- The end-game critical path is the PSUM-evacuation pipeline: DVE/ACT
  are the only PSUM readers (GPSIMD has no PSUM port) at 1 elem/lane/
  cycle, so 4.2M f32 psum values cost ~19us across both engines. PSUM
  is 2-bank tiles [128, 2, 512]; one [128,1024] copy per engine per
  group; the ~10%-faster scalar engine takes one extra pair.
- DMA: strict FIFO on the Sync/SP HWDGE ring (~420 GB/s reads, ~360-400
  writes): a 64KB lead weight slice (groups 0-1), the first input
  units, the remaining weights, the rest of the inputs — so the copy
  pipeline starts ~11.8us — then the output DMAs. Outputs are QUAD-
  major (4 groups, 8KB bf16 rows) to cut issue count and descriptor
  overhead; the final two groups ship as just 2 DMAs (fp8 first) so the
  exit-gating bytes leave right after the last copies. The ACT ring is
  unused for DMA (it starts ~4us late on first use).
- HAM pre-warm garbage MMs bridge until the first input unit lands;
  keep-warm fillers guard against input jitter re-throttling the PE.
- Fixed overheads bound the total: ~8us NRT preamble to first DMA byte,
  ~8.7us tail (completion receipt + barriers + NRT's 51-sems/engine
  postamble reset). Measured 46.2/47.4us (run-to-run throttle noise
  ~1-7us; one run measured 53.8 with everything uniformly ~1.2x slow).
"""

import numpy as np
import ml_dtypes

B_TOTAL = 8192
D_IN = 4096
D_OUT = 4096
BLK = 128
K_IN = D_IN // BLK    # 32
K_OUT = D_OUT // BLK  # 32
N_CORES = 8
B_SHARD = B_TOTAL // N_CORES  # 1024
NG = 16               # groups of 4 bins (64 plane-pairs)
NU = 8                # input DMA units (2 groups = 0.5MB each)
NQ = 4                # output quads (4 groups each)
MM_FREE = 512         # moving free dim per matmul (one PSUM bank)
XSCALE_TGT = 14.0     # fp8 e3m4 max normal is 15.5
YSCALE_TGT = 12.9

_compiled = None


def _build_module():
    import concourse.bass as bass
    import concourse.tile as tile
    from concourse import bacc, mybir

    nc = bacc.Bacc("TRN2", target_bir_lowering=False, debug=False)

    bf = mybir.dt.bfloat16
    f8 = mybir.dt.float8e3
    f32 = mybir.dt.float32

    # xq[unit, p, gi, u, m]: p = 64*half + 32*comp + j; slot = (g, half, u)
    xq = nc.dram_tensor("xq", [NU, 128, 2, 2, B_SHARD], f8, kind="ExternalInput")
    # weights, lhsT of the 64x64 bin matrix; lead slice = groups 0-1 so the
    # first MMs (and the copy pipeline) gate on a 64KB DMA, not 512KB
    wl = nc.dram_tensor("wl", [128, 2, 2, 64], bf, kind="ExternalInput")
    wr = nc.dram_tensor("wr", [128, NG - 2, 2, 64], bf, kind="ExternalInput")
    # outputs, QUAD-major: [qu, p, gi(4), mc, m']; p = 64*oh + 32*comp + o;
    # bin slot (g = 4*qu + gi, half, u=oh); bank A (half0) bf16, bank B fp8
    ybf = nc.dram_tensor("ybf", [NQ, 128, 4, 2, MM_FREE], bf, kind="ExternalOutput")
    yq8 = nc.dram_tensor("yq8", [NQ, 128, 4, 2, MM_FREE], f8, kind="ExternalOutput")

    U_ELEMS = 128 * 2 * 2 * B_SHARD   # 0.5MB fp8 per input unit
    QO_ELEMS = 128 * 4 * 2 * MM_FREE  # per output quad per tensor

    with tile.TileContext(nc) as tc:
        with (
            tc.tile_pool(name="sb", bufs=1) as spool,
            tc.tile_pool(name="psum", bufs=2, space="PSUM") as ppool,
        ):
            w_lead = spool.tile([128, 2, 2, 64], bf, name="wl")
            w_rest = spool.tile([128, NG - 2, 2, 64], bf, name="wr")
            nc.sync.dma_start(w_lead[:], wl[:])

            scratch = spool.tile([128, MM_FREE], bf, name="scratch")
            nc.vector.memset(scratch[:], 0.0)

            xts = []
            for ui in range(NU):
                xt = spool.tile(
                    [128, 2, 2, B_SHARD], f8, tag="xt", name=f"xt{ui}", bufs=NU
                )
                nc.sync.dma_start(
                    xt[:],
                    bass.AP(xq, ui * U_ELEMS, [[2 * 2 * B_SHARD, 128], [1, 2 * 2 * B_SHARD]]),
                )
                xts.append(xt)
                if ui == 1:
                    nc.sync.dma_start(w_rest[:], wr[:])

            otA = otB = None
            for g in range(NG):
                xt, gi = xts[g // 2], g % 2
                wti, wg = (w_lead, g) if g < 2 else (w_rest, g - 2)
                psA = ppool.tile([128, 2, MM_FREE], f32, tag="psA", name=f"psA_{g}")
                psB = ppool.tile([128, 2, MM_FREE], f32, tag="psB", name=f"psB_{g}")
                if g == 0:
                    # HAM pre-warm: garbage matmuls while the lead weights and
                    # first input unit stream in (~3.4us of sustained activity
                    # trips the 2.4GHz un-throttle); wiped by start=True later
                    for k in range(11):
                        nc.tensor.matmul(
                            (psA if k % 2 == 0 else psB)[:, k % 2, :],
                            lhsT=scratch[:, 0:128],
                            rhs=scratch[:],
                            start=True,
                            stop=True,
                        )
                elif g % 2 != 0 and g < NG - 2:
                    # keep-warm filler; insurance against input-DMA jitter
                    nc.tensor.matmul(
                        psA[:, 0, :],
                        lhsT=scratch[:, 0:128],
                        rhs=scratch[:],
                        start=True,
                        stop=True,
                    )
                for mc in range(2):
                    s = slice(mc * MM_FREE, (mc + 1) * MM_FREE)
                    for half, u, ps, tp in (
                        (0, 0, psA, (0, 0)),
                        (0, 1, psA, (0, 64)),
                        (1, 0, psB, (64, 0)),
                        (1, 1, psB, (64, 64)),
                    ):
                        rows = slice(64 * half, 64 * half + 64)
                        cols = slice(tp[1], tp[1] + 64)
                        nc.tensor.matmul(
                            ps[cols, mc, :],
                            lhsT=wti[rows, wg, u, :],
                            rhs=xt[rows, gi, u, s],
                            start=True,
                            stop=True,
                            tile_position=tp,
                        )

                qu, qi = g // 4, g % 4
                if g < NG - 4:
                    if qi == 0:
                        otA = spool.tile([128, 4, 2, MM_FREE], bf, tag="otA", name=f"otA{qu}", bufs=NQ - 1)
                        otB = spool.tile([128, 4, 2, MM_FREE], f8, tag="otB", name=f"otB{qu}", bufs=NQ - 1)
                    oA, oB = otA[:, qi], otB[:, qi]
                else:
                    # last quad split into two pair tiles: (12,13) ships as a
                    # normal pair; (14,15) is the exit-gating final pair
                    if qi % 2 == 0:
                        otA = spool.tile([128, 2, 2, MM_FREE], bf, tag="otA2", name=f"otA2_{qi}", bufs=2)
                        otB = spool.tile([128, 2, 2, MM_FREE], f8, tag="otB2", name=f"otB2_{qi}", bufs=2)
                    oA, oB = otA[:, qi % 2], otB[:, qi % 2]
                # evacuate PSUM: one [128,1024] copy per engine per group;
                # scalar (1.11us/copy) takes one extra pair vs vector (1.22)
                if g == 7:
                    nc.scalar.copy(oA, psA[:])
                    nc.scalar.copy(oB, psB[:])
                else:
                    nc.vector.tensor_copy(oA, psA[:])
                    nc.scalar.copy(oB, psB[:])

                if g < NG - 4:
                    if qi == 3:
                        nc.sync.dma_start(
                            bass.AP(ybf, qu * QO_ELEMS, [[4096, 128], [1, 4096]]),
                            otA[:],
                        )
                        nc.sync.dma_start(
                            bass.AP(yq8, qu * QO_ELEMS, [[4096, 128], [1, 4096]]),
                            otB[:],
                        )
                elif qi == 1:
                    nc.sync.dma_start(
                        bass.AP(ybf, 3 * QO_ELEMS, [[4096, 128], [1, 2048]]),
                        otA[:],
                    )
                    nc.sync.dma_start(
                        bass.AP(yq8, 3 * QO_ELEMS, [[4096, 128], [1, 2048]]),
                        otB[:],
                    )
                elif qi == 3:
                    # final pair: fp8 first so the slower small-row stream
                    # overlaps the bf16 issue; only these 2 DMAs gate exit
                    nc.sync.dma_start(
                        bass.AP(yq8, 3 * QO_ELEMS + 2048, [[4096, 128], [1, 2048]]),
                        otB[:],
                    )
                    nc.sync.dma_start(
                        bass.AP(ybf, 3 * QO_ELEMS + 2048, [[4096, 128], [1, 2048]]),
                        otA[:],
                    )

    nc.compile()
    return nc


def _get_module():
    global _compiled
    if _compiled is None:
        _compiled = _build_module()
    return _compiled


def kernel(x: np.ndarray, W: np.ndarray, D_bernoulli: np.ndarray) -> np.ndarray:
    from concourse.bass_utils import run_bass_kernel_spmd

    bf16 = ml_dtypes.bfloat16
    e3m4 = ml_dtypes.float8_e3m4
    x = np.asarray(x, dtype=np.float32)
    W = np.asarray(W, dtype=np.float32)
    D = np.asarray(D_bernoulli, dtype=np.float32)

    # --- host: forward rfft of (x*D) blocks ---
    xd = (x * D[None, :]).reshape(B_TOTAL, K_IN, BLK)
    Xf = np.fft.rfft(xd, axis=-1)                 # [B, 32, 65]
    Xr = np.ascontiguousarray(Xf.real.transpose(2, 1, 0)).astype(np.float32)  # [65, 32, B]
    Xi = np.ascontiguousarray(Xf.imag.transpose(2, 1, 0)).astype(np.float32)
    XR = Xr[:64]                                  # [64 bins, 32 j, B]
    XI = Xi[:64].copy()
    XI[0] = Xr[64]                                # R64 rides in the I0 slot

    # fp8 e3m4 input scale; the weights absorb 1/s
    s = XSCALE_TGT / max(np.abs(XR).max(), np.abs(XI).max())
    XRq = (XR * s).astype(e3m4)
    XIq = (XI * s).astype(e3m4)

    # --- host: 64x64 bin matrices M2 = [[A,C],[B,D]] (lhsT) ---
    Vf = np.conj(np.fft.rfft(W, axis=-1))         # [o, j, 65]
    VR = np.ascontiguousarray(Vf.real.transpose(2, 1, 0)).astype(np.float32)  # [65, j, o]
    VI = np.ascontiguousarray(Vf.imag.transpose(2, 1, 0)).astype(np.float32)
    M2 = np.empty((64, 64, 64), dtype=np.float32)  # [bin, jc, oc]
    M2[:, :K_IN, :K_OUT] = VR[:64]                 # A  (YR += A.XR)
    M2[:, :K_IN, K_OUT:] = VI[:64]                 # C  (YI += C.XR)
    M2[:, K_IN:, :K_OUT] = -VI[:64]                # B  (YR += B.XI)
    M2[:, K_IN:, K_OUT:] = VR[:64]                 # D  (YI += D.XI)
    M2[0, :K_IN, K_OUT:] = 0.0                     # bin 0/64 are real-only
    M2[0, K_IN:, :K_OUT] = 0.0
    M2[0, K_IN:, K_OUT:] = VR[64]                  # R64 channel in the I0 slot

    # --- bin permutation: 32 lowest-energy bins -> fp8 output half ---
    XRf = XRq.astype(np.float32)
    XIf = XIq.astype(np.float32)
    PX = np.concatenate(
        [(XRf ** 2).mean(axis=2), (XIf ** 2).mean(axis=2)], axis=1
    )                                              # [64, 64] E[x2q^2] per jc
    proxy = np.einsum('fjo,fj->f', M2 ** 2, PX) / (s * s)
    wgt = np.full(64, 2.0); wgt[0] = 1.0
    order = np.argsort(proxy * wgt)
    lo_bins = np.sort(order[:32])                  # fp8 output half (half=1)
    hi_bins = np.sort(order[32:])                  # bf16 output half (half=0)
    slot_bin = np.empty((NG, 2, 2), dtype=np.int64)
    slot_bin[:, 0, :] = hi_bins.reshape(NG, 2)
    slot_bin[:, 1, :] = lo_bins.reshape(NG, 2)

    # --- per-bin output scales for the fp8 half (hard C-S bound) ---
    x2n = np.sqrt(
        (XRf ** 2).sum(axis=1) + (XIf ** 2).sum(axis=1)
    ).max(axis=1)                                  # [64] max_b ||x2q_b||
    coln = np.sqrt((M2 ** 2).sum(axis=1)).max(axis=1) / s   # [64] max_oc ||.||
    bound = coln * x2n                             # per-bin hard |psum| bound
    sigma = np.ones(64, dtype=np.float32)
    sigma[lo_bins] = YSCALE_TGT / bound[lo_bins]

    # --- weights -> wt[p, g, u, oc] per the slot map, scaled ---
    M2s = M2 * (sigma / s)[:, None, None]
    wt_host = np.empty((128, NG, 2, 64), dtype=bf16)
    for g in range(NG):
        for half in range(2):
            for u in range(2):
                wt_host[64 * half : 64 * half + 64, g, u, :] = (
                    M2s[slot_bin[g, half, u]].astype(bf16)
                )
    wl_host = np.ascontiguousarray(wt_host[:, :2])
    wr_host = np.ascontiguousarray(wt_host[:, 2:])

    # --- pack inputs per the slot map: xq[un, 64h+32c+j, gi, u, m] ---
    Z = np.stack([XRq, XIq], axis=1)               # [64, 2(comp), 32, B]
    xq_all = np.empty((NU, 2, 2, K_IN, 2, 2, B_TOTAL), dtype=e3m4)
    # axes: [un, half, comp, j, gi, u, m]
    for g in range(NG):
        for half in range(2):
            for u in range(2):
                xq_all[g // 2, half, :, :, g % 2, u, :] = Z[slot_bin[g, half, u]]
    xq_all = xq_all.reshape(NU, 128, 2, 2, B_TOTAL)

    in_maps = []
    for c in range(N_CORES):
        sl = slice(c * B_SHARD, (c + 1) * B_SHARD)
        in_maps.append(
            {"xq": np.ascontiguousarray(xq_all[:, :, :, :, sl]),
             "wl": wl_host, "wr": wr_host}
        )

    nc = _get_module()
    res = run_bass_kernel_spmd(nc, in_maps, core_ids=list(range(N_CORES)))

    # --- host: unpack spectra, irfft, reassemble ---
    inv_sigma = (1.0 / sigma).astype(np.float32)
    out = np.empty((B_TOTAL, D_OUT), dtype=np.float32)
    NB = BLK // 2 + 1
    for c in range(N_CORES):
        PS = np.empty((64, 2, K_OUT, B_SHARD), dtype=np.float32)  # [bin, comp, o, m]
        for half, key in ((0, "ybf"), (1, "yq8")):
            y = np.asarray(res.results[c][key], dtype=np.float32)  # [NQ,128,4,2,512]
            # y[qu, 64*oh + 32*comp + o, gi, mc, m']; slot (g=4qu+gi, half, u=oh)
            yb = y.reshape(NQ, 2, 2, K_OUT, 4, 2 * MM_FREE)  # [qu, oh, comp, o, gi, m]
            for qu in range(NQ):
                for gi in range(4):
                    for oh in range(2):
                        b = slot_bin[4 * qu + gi, half, oh]
                        PS[b] = yb[qu, oh, :, :, gi, :] * inv_sigma[b]
        psR, psI = PS[:, 0], PS[:, 1]                    # [64, o, m]
        Yf = np.zeros((B_SHARD, K_OUT, NB), dtype=np.complex64)
        Yf[:, :, :64] = (psR + 1j * psI).transpose(2, 1, 0)
        Yf[:, :, 0] = psR[0].T
        Yf[:, :, 64] = psI[0].T
        ob = np.fft.irfft(Yf, n=BLK, axis=-1)            # [m, 32, 128]
        out[c * B_SHARD : (c + 1) * B_SHARD] = ob.reshape(B_SHARD, D_OUT)
    return out
